# revision 1
# baseline (speedup 1.0000x reference)
"""Trainium2 Bass kernel for nn_Net_71270687310327 (scatter_memory).

Computation (see reference):
  - keys = (timings+1)*512 + slot_index, with argmin(surprise*0.9) slot's key
    overridden to its slot index (forces rank 0, stable-sort tiebreak exact).
  - rank[b,m] = #{m' : key[b,m'] < key[b,m]}  (all keys distinct)
  - pred_in = [sorted memory rows | timing bits], fed to a 4-layer MLP.

Sharding: W0 row-sharded over 8 cores by slot-rank range (64 ranks/core,
17024 rows of W0 each, fully contiguous HBM reads). Each core gathers only
its 64 ranks' memory rows (dma_gather), computes a partial h = pred_in @ W0
contribution, AllReduce over the 8 cores, then every core redundantly runs
the small W1/W2/Wout layers.

The same program runs on all 8 cores (SPMD); all per-core differences are
carried by per-core input constants (W0 shard, rank-range constants).
"""

import sys, os

sys.path.insert(0, "/opt/trn_rl_repo")

import numpy as np

import concourse.bass as bass
import concourse.bacc as bacc
import concourse.mybir as mybir
from concourse import tile
from concourse import bass_utils

class _SkipRest(Exception):
    pass


F32 = mybir.dt.float32
I16 = mybir.dt.int16
ALU = mybir.AluOpType
ACTF = mybir.ActivationFunctionType

B, M, V, H, TD = 32, 512, 256, 1024, 10
NC = 8
RPC = M // NC            # 64 ranks per core
MEMROWS = B * M          # 16384
MEMP = MEMROWS + B       # 16416 (gather source rows: memory rows + x rows)
NKT = RPC * V // 128     # 128 mem k-tiles per core
NBT = RPC * TD // 128    # 5 bits k-tiles per core
W0S_ROWS = RPC * V + RPC * TD  # 17024
NIDX = RPC * B           # 2048 gather indices per core


def build_program(stage="full"):
    nkt_lim = NKT + NBT
    if stage.startswith("parth") and stage != "parth":
        nkt_lim = int(stage[5:])
        stage = "parth"
    lvl = {"idx": 0, "tk": 1, "parth": 2, "full": 3}[stage]
    nc = bacc.Bacc(
        "TRN2",
        target_bir_lowering=False,
        debug=False,
        enable_asserts=False,
        num_devices=NC,
    )

    def din(name, shape, dtype=F32):
        return nc.dram_tensor(name, list(shape), dtype, kind="ExternalInput").ap()

    mem_plus = din("mem_plus", (MEMP, V))
    timings = din("timings", (B, M))
    msur = din("msur", (B, M))
    w0s = din("W0s", (W0S_ROWS, H))
    w1 = din("W1", (H, H))
    w2 = din("W2", (H, H))
    wout = din("Wout", (H, V))
    b0r = din("b0r", (B, H))
    b1r = din("b1r", (B, H))
    b2r = din("b2r", (B, H))
    boutr = din("boutr", (B, V))
    c_eye = din("c_eye", (128, 128))
    c_esel = din("c_esel", (B, B * 128))
    c_iota = din("c_iota512", (B, M))
    c_iotam = din("c_iotam", (128, 4))
    c_rrow = din("c_rrow", (128, RPC))
    c_sel16 = din("c_sel16", (1, 16 * 128))
    c_amask = din("c_amask", (128, 128))
    c_coff = din("c_coff", (128, 128))
    c_rtd = din("c_rtd", (RPC, NBT * TD * 128))

    out = nc.dram_tensor("out", [B, V], F32, kind="ExternalOutput").ap()
    dbg = (nc.dram_tensor("dbg", [128, 256], F32, kind="ExternalOutput").ap()
           if stage != "full" else None)

    with tile.TileContext(nc) as tc:
        with (
            tc.tile_pool(name="const", bufs=1) as constp,
            tc.tile_pool(name="state", bufs=1) as state,
            tc.tile_pool(name="wres", bufs=1) as wres,
            tc.tile_pool(name="krep", bufs=2) as krepp,
            tc.tile_pool(name="pt", bufs=8) as ptp,
            tc.tile_pool(name="w0t", bufs=6) as w0p,
            tc.tile_pool(name="pk", bufs=1, space="PSUM") as pkp,
            tc.tile_pool(name="pflat", bufs=1, space="PSUM") as pflatp,
            tc.tile_pool(name="psort", bufs=1, space="PSUM") as psortp,
            tc.tile_pool(name="ptr", bufs=2, space="PSUM") as ptrp,
            tc.tile_pool(name="ph", bufs=1, space="PSUM") as php,
            tc.tile_pool(name="dram", bufs=1, space="DRAM") as dramp,
        ):
            # ---- constants / small state into SBUF
            def load(pool, ap):
                t = pool.tile(list(ap.shape), ap.dtype, tag=f"ld_{ap.tensor.name}")
                nc.sync.dma_start(t[:], ap)
                return t

            eye = load(constp, c_eye)
            esel = load(constp, c_esel)
            iota = load(constp, c_iota)
            iotam = load(constp, c_iotam)
            rrow = load(constp, c_rrow)
            sel16 = load(constp, c_sel16)
            amask = load(constp, c_amask)
            coff = load(constp, c_coff)
            rtd = load(constp, c_rtd)
            b0s = load(constp, b0r)
            b1s = load(constp, b1r)
            b2s = load(constp, b2r)
            bouts = load(constp, boutr)
            t_sb = load(state, timings)
            ms_sb = load(state, msur)

            # resident output-layer weights; W1/W2 stream through the k-tile pool
            wos = wres.tile([128, 8 * V], F32, tag="wos")
            for kt in range(8):
                nc.sync.dma_start(wos[:, kt * V:(kt + 1) * V], wout[kt * 128:(kt + 1) * 128, :])

            # ---- stage A: keys -------------------------------------------
            msur2 = state.tile([B, M], F32, tag="msur2")
            nc.vector.tensor_scalar(msur2[:], ms_sb[:], 0.9, None, ALU.mult)
            minv = state.tile([B, 1], F32, tag="minv")
            nc.vector.tensor_reduce(minv[:], msur2[:], axis=mybir.AxisListType.X, op=ALU.min)
            mask = state.tile([B, M], mybir.dt.uint8, tag="mask")
            nc.vector.tensor_scalar(mask[:], msur2[:], minv[:], None, ALU.is_equal)
            cand = state.tile([B, M], F32, tag="cand")
            nc.vector.memset(cand[:], 1.0e9)
            nc.vector.copy_predicated(cand[:], mask[:], iota[:])
            idx0 = state.tile([B, 1], F32, tag="idx0")
            nc.vector.tensor_reduce(idx0[:], cand[:], axis=mybir.AxisListType.X, op=ALU.min)

            keys = state.tile([B, M], F32, tag="keys")
            # (t+1)*512 + m  =  t*512 + 512 + m
            nc.vector.tensor_scalar(keys[:], t_sb[:], 512.0, 512.0, ALU.mult, ALU.add)
            nc.vector.tensor_tensor(keys[:], keys[:], iota[:], ALU.add)
            mask2 = state.tile([B, M], mybir.dt.uint8, tag="mask2")
            nc.vector.tensor_scalar(mask2[:], iota[:], idx0[:], None, ALU.is_equal)
            nc.vector.copy_predicated(keys[:], mask2[:], iota[:])

            # ---- keysT via PE transpose ----------------------------------
            keysT = state.tile([128, 4 * B], F32, tag="keysT")
            for mt in range(4):
                ptt = ptrp.tile([128, 128], F32, tag="pm")
                nc.tensor.transpose(ptt[:, 0:B], keys[:, mt * 128:(mt + 1) * 128], eye[0:B, 0:B])
                nc.scalar.activation(keysT[:, mt * B:(mt + 1) * B], ptt[:, 0:B], ACTF.Copy)

            # ---- ranks, P^T, order/sorted extraction ---------------------
            rank_sb = state.tile([128, 4 * B], F32, tag="rank")
            scratch = state.tile([128, M], F32, tag="scratch")
            flat = state.tile([1, NIDX], F32, tag="flat")
            psort_t = psortp.tile([RPC, B], F32, tag="psort")
            for g in range(4):
                pflat_t = pflatp.tile([1, 8 * RPC], F32, tag="pflat")
                for b8 in range(8):
                    b = g * 8 + b8
                    pk_t = pkp.tile([128, M], F32, tag="pkrep")
                    nc.tensor.matmul(pk_t[:], esel[:, b * 128:(b + 1) * 128], keys[:],
                                     start=True, stop=True)
                    krep = krepp.tile([128, M], F32, tag="krep")
                    nc.scalar.activation(krep[:], pk_t[:], ACTF.Copy)
                    for mt in range(4):
                        nc.vector.tensor_scalar(
                            scratch[:], krep[:], keysT[:, mt * B + b:mt * B + b + 1], None,
                            ALU.is_lt, ALU.add,
                            accum_out=rank_sb[:, b * 4 + mt:b * 4 + mt + 1])
                    pts = []
                    for mt in range(4):
                        pt_t = ptp.tile([128, RPC], F32, tag="pt")
                        nc.vector.tensor_scalar(
                            pt_t[:], rrow[:], rank_sb[:, b * 4 + mt:b * 4 + mt + 1], None,
                            ALU.is_equal)
                        pts.append(pt_t)
                    for mt in range(4):
                        nc.tensor.matmul(
                            pflat_t[0:1, b8 * RPC:(b8 + 1) * RPC],
                            iotam[:, mt:mt + 1], pts[mt][:],
                            start=(mt == 0), stop=(mt == 3))
                        nc.tensor.matmul(
                            psort_t[0:RPC, b:b + 1],
                            pts[mt][:], keysT[:, mt * B + b:mt * B + b + 1],
                            start=(mt == 0), stop=(mt == 3))
                nc.scalar.activation(flat[0:1, g * 512:(g + 1) * 512], pflat_t[:], ACTF.Copy)

            # ---- bits from sorted keys -----------------------------------
            # binary decomposition of sorted key (< 2^19); timing bit d of t
            # is key bit d+9.  u_all[:, d*B:(d+1)*B] = bit (d+9) of key.
            skT = state.tile([RPC, B], F32, tag="skT")
            nc.scalar.activation(skT[:], psort_t[:], ACTF.Copy)
            rem = state.tile([RPC, B], F32, tag="rem")
            nc.vector.tensor_copy(rem[:], skT[:])
            u_all = state.tile([RPC, TD * B], F32, tag="u_all")
            tmpu = state.tile([RPC, B], F32, tag="tmpu")
            for j in range(18, 8, -1):
                d = j - 9
                ud = u_all[:, d * B:(d + 1) * B]
                nc.vector.tensor_scalar(ud, rem[:], float(2 ** j), None, ALU.is_ge)
                nc.vector.tensor_scalar(tmpu[:], ud, float(2 ** j), None, ALU.mult)
                nc.vector.tensor_tensor(rem[:], rem[:], tmpu[:], ALU.subtract)
            # bits_sb[t][p, b] = u_{d(p)}[r(p), b] via selection matmuls
            bits_sb = state.tile([128, NBT * B], F32, tag="bits")
            for t in range(NBT):
                pb = ptrp.tile([128, 128], F32, tag="pm")
                for d in range(TD):
                    nc.tensor.matmul(pb[:, 0:B],
                                     rtd[:, (t * TD + d) * 128:(t * TD + d + 1) * 128],
                                     u_all[:, d * B:(d + 1) * B],
                                     start=(d == 0), stop=(d == TD - 1))
                nc.scalar.activation(bits_sb[:, t * B:(t + 1) * B], pb[:, 0:B], ACTF.Copy)

            # ---- gather indices ------------------------------------------
            pidx_t = ptrp.tile([128, 128], F32, tag="pm")
            flat_v = flat.rearrange("p (n s) -> p n s", s=16)
            for k in range(16):
                nc.tensor.matmul(pidx_t[:], sel16[0:1, k * 128:(k + 1) * 128],
                                 flat_v[:, :, k], start=(k == 0), stop=(k == 15))
            tmpidx = state.tile([128, 128], F32, tag="tmpidx")
            nc.vector.tensor_tensor(tmpidx[:], pidx_t[:], amask[:], ALU.mult)
            idx_sb = state.tile([128, 128], I16, tag="idx")
            nc.vector.tensor_tensor(idx_sb[:], tmpidx[:], coff[:], ALU.add)

            if stage == "idx":
                nc.vector.tensor_copy(tmpidx[:], idx_sb[:])
                nc.sync.dma_start(dbg[:, 0:128], tmpidx[:])
                nc.sync.dma_start(dbg[:, 128:256], bits_sb[:, 0:128])
            do_rest = lvl >= 1
            try:
              if not do_rest:
                  raise _SkipRest
              # ---- gather + transpose to pred_in^T tiles -------------------
              G = state.tile([128, 16 * V], F32, tag="G")
              nc.gpsimd.dma_gather(
                  out_ap=G.rearrange("p (c e) -> p c e", e=V),
                  in_ap=mem_plus,
                  idxs_ap=idx_sb[:],
                  num_idxs=NIDX,
                  num_idxs_reg=NIDX,
                  elem_size=V,
                  single_packet=False,
              )
              T_all = state.tile([128, 16 * V], F32, tag="T_all")
              for c in range(16):
                  for hh in range(2):
                      off = c * V + hh * 128
                      pt2 = ptrp.tile([128, 128], F32, tag="pm")
                      nc.tensor.transpose(pt2[:], G[:, off:off + 128], eye[:])
                      nc.scalar.activation(T_all[:, off:off + 128], pt2[:], ACTF.Copy)

              # ---- repack transposed tiles to k-tile-major contiguous ------
              # T_all col = 256*cb + 128*h + 64*b2 + r  ->  TK col = 64*r + 32*h + 2*cb + b2
              TK = state.tile([128, 16 * V], F32, tag="TK")
              t_in = T_all.rearrange("p (cb h b2 r) -> p r h cb b2", cb=16, h=2, b2=2, r=RPC)
              tk_out = TK.rearrange("p (r h cb b2) -> p r h cb b2", r=RPC, h=2, cb=16, b2=2)
              nc.vector.tensor_copy(tk_out[:], t_in[:])

              if stage == "tk":
                  nc.sync.dma_start(dbg[:, 0:256], TK[:, 0:256])
              if lvl < 2:
                  raise _SkipRest
              # ---- main matmul: partial h = pred_in_shard @ W0_shard -------
              ph_t = php.tile([B, H], F32, tag="ph")
              for kt in range(nkt_lim):
                  w0t = w0p.tile([128, H], F32, tag="w0t")
                  nc.sync.dma_start(w0t[:], w0s[kt * 128:(kt + 1) * 128, :])
                  if kt < NKT:
                      lhsT = TK[:, kt * B:(kt + 1) * B]
                  else:
                      tb = kt - NKT
                      lhsT = bits_sb[:, tb * B:(tb + 1) * B]
                  last = kt == nkt_lim - 1
                  nc.tensor.matmul(ph_t[:, 0:512], lhsT, w0t[:, 0:512],
                                   start=(kt == 0), stop=last)
                  nc.tensor.matmul(ph_t[:, 512:1024], lhsT, w0t[:, 512:1024],
                                   start=(kt == 0), stop=last)

              # ---- AllReduce partial h over the 8 cores --------------------
              part_h = state.tile([B, H], F32, tag="part_h")
              nc.vector.tensor_copy(part_h[:], ph_t[:])
              if stage == "parth":
                  nc.sync.dma_start(dbg[0:B, 0:256], part_h[:, 0:256])
              if lvl < 3:
                  raise _SkipRest
              cc_in = dramp.tile([B, H], F32, tag="cc_in")
              cc_out = dramp.tile([B, H], F32, tag="cc_out")
              nc.sync.dma_start(cc_in[:], part_h[:])
              nc.gpsimd.collective_compute(
                  "AllReduce", ALU.add,
                  replica_groups=[list(range(NC))],
                  ins=[cc_in.opt()],
                  outs=[cc_out.opt()],
              )
              h_sb = state.tile([B, H], F32, tag="h_sb")
              nc.sync.dma_start(h_sb[:], cc_out[:])

              # ---- dense layers (replicated on every core) -----------------
              nc.vector.tensor_tensor(h_sb[:], h_sb[:], b0s[:], ALU.add)
              nc.vector.tensor_scalar(h_sb[:], h_sb[:], 0.0, None, ALU.max)

              def dense(h_in, w_dram, w_sb, bias_sb, n_out, relu, tag):
                  hT = state.tile([128, 8 * B], F32, tag=f"hT_{tag}")
                  for kt in range(8):
                      ptt = ptrp.tile([128, 128], F32, tag="pm")
                      nc.tensor.transpose(ptt[:, 0:B], h_in[:, kt * 128:(kt + 1) * 128], eye[0:B, 0:B])
                      nc.scalar.activation(hT[:, kt * B:(kt + 1) * B], ptt[:, 0:B], ACTF.Copy)
                  pho = php.tile([B, n_out], F32, tag="ph")
                  for kt in range(8):
                      if w_dram is not None:
                          wt = w0p.tile([128, H], F32, tag="w0t")
                          nc.sync.dma_start(wt[:, 0:n_out], w_dram[kt * 128:(kt + 1) * 128, :])
                      else:
                          wt = None
                      for j0 in range(0, n_out, 512):
                          jn = min(512, n_out - j0)
                          rhs = (wt[:, j0:j0 + jn] if wt is not None
                                 else w_sb[:, kt * n_out + j0:kt * n_out + j0 + jn])
                          nc.tensor.matmul(
                              pho[:, j0:j0 + jn], hT[:, kt * B:(kt + 1) * B], rhs,
                              start=(kt == 0), stop=(kt == 7))
                  h_next = state.tile([B, n_out], F32, tag=f"h_{tag}")
                  nc.vector.tensor_tensor(h_next[:], pho[:], bias_sb[:], ALU.add)
                  if relu:
                      nc.vector.tensor_scalar(h_next[:], h_next[:], 0.0, None, ALU.max)
                  return h_next

              h1 = dense(h_sb, w1, None, b1s, H, True, "l1")
              h2 = dense(h1, w2, None, b2s, H, True, "l2")
              logits = dense(h2, None, wos, bouts, V, False, "lo")
              nc.sync.dma_start(out, logits[:])
            except _SkipRest:
                pass

    nc.compile()
    return nc


def make_in_maps(inputs):
    x = np.asarray(inputs["x"], np.float32)
    memory = np.asarray(inputs["memory"], np.float32)
    timings = np.asarray(inputs["memory_timings"], np.float32)
    msur = np.asarray(inputs["memory_surprise"], np.float32)
    W0 = np.asarray(inputs["W0"], np.float32)
    W1 = np.asarray(inputs["W1"], np.float32)
    W2 = np.asarray(inputs["W2"], np.float32)
    Wout = np.asarray(inputs["Wout"], np.float32)
    b0 = np.asarray(inputs["b0"], np.float32)
    b1 = np.asarray(inputs["b1"], np.float32)
    b2 = np.asarray(inputs["b2"], np.float32)
    bout = np.asarray(inputs["bout"], np.float32)

    mem_plus = np.concatenate([memory.reshape(MEMROWS, V), x], 0)

    # shared constants
    eye = np.eye(128, dtype=np.float32)
    esel = np.zeros((B, B * 128), np.float32)
    for b in range(B):
        esel[b, b * 128:(b + 1) * 128] = 1.0
    iota512 = np.broadcast_to(np.arange(M, dtype=np.float32), (B, M)).copy()
    iotam = np.empty((128, 4), np.float32)
    for mt in range(4):
        iotam[:, mt] = np.arange(128) + mt * 128
    sel16 = np.zeros((1, 16 * 128), np.float32)
    for k in range(16):
        p = np.arange(128)
        sel16[0, k * 128:(k + 1) * 128] = (p % 16 == k)
    rtd = np.zeros((RPC, NBT * TD * 128), np.float32)
    for t in range(NBT):
        for p in range(128):
            l = t * 128 + p
            rp, dp = l // TD, l % TD
            rtd[rp, (t * TD + dp) * 128 + p] = 1.0

    shared = {
        "mem_plus": mem_plus,
        "timings": timings,
        "msur": msur,
        "W1": W1, "W2": W2, "Wout": Wout,
        "b0r": np.broadcast_to(b0, (B, H)).copy(),
        "b1r": np.broadcast_to(b1, (B, H)).copy(),
        "b2r": np.broadcast_to(b2, (B, H)).copy(),
        "boutr": np.broadcast_to(bout, (B, V)).copy(),
        "c_eye": eye, "c_esel": esel, "c_iota512": iota512,
        "c_iotam": iotam, "c_sel16": sel16, "c_rtd": rtd,
    }

    in_maps = []
    p = np.arange(128)
    f = np.arange(128)
    ii = 16 * f[None, :] + (p % 16)[:, None]   # [128,128] linear gather positions
    bb = ii // RPC
    rr = ii % RPC
    for core in range(NC):
        w0shard = np.concatenate(
            [W0[core * RPC * V:(core + 1) * RPC * V],
             W0[M * V + core * RPC * TD: M * V + (core + 1) * RPC * TD]], 0)
        rrowc = np.broadcast_to(
            np.arange(core * RPC, (core + 1) * RPC, dtype=np.float32), (128, RPC)).copy()
        am = np.ones((128, 128), np.float32)
        co = (512.0 * bb).astype(np.float32)
        if core == 0:
            r0 = rr == 0
            am[r0] = 0.0
            co[r0] = (MEMROWS + bb)[r0]
        m = dict(shared)
        m["W0s"] = np.ascontiguousarray(w0shard)
        m["c_rrow"] = rrowc
        m["c_amask"] = am
        m["c_coff"] = co
        in_maps.append(m)
    return in_maps


_NC_CACHE = None


def kernel(**inputs) -> np.ndarray:
    global _NC_CACHE
    if _NC_CACHE is None:
        _NC_CACHE = build_program()
    nc = _NC_CACHE
    in_maps = make_in_maps(inputs)
    res = bass_utils.run_bass_kernel_spmd(nc, in_maps, core_ids=list(range(NC)))
    return np.asarray(res.results[0]["out"], np.float32)


if __name__ == "__main__":
    np.random.seed(0)
    build_program()
    print("build OK")



# revision 16
# speedup vs baseline: 1.8426x; 1.8426x over previous
"""Trainium2 Bass kernel for nn_Net_71270687310327 (scatter_memory).

Computation (see reference): argmin-scatter into memory, stable sort by
timings, gather sorted rows + timing bits, 4-layer MLP.

Design (v2):
  - keys = (t+1)*512 + m, argmin slot overridden to key=m (rank 0).
  - ranks by brute-force count, split across DVE (is_lt+accum) and the
    Activation engine (Sign trick: rank = (sum sign(k - k') + 511)/2),
    reading the per-batch broadcast keys (krep) directly from PSUM.
  - gpsimd local_scatter inverts the permutation: scatter gather-row ids
    (512b+m, x-row override baked in) and effective timings to positions
    rank-64c (out-of-window ranks -> negative idx, dropped).
  - gpsimd dma_gather(transpose=True) on bf16 mem_plus directly yields
    all 128 transposed lhsT k-tiles (no PE transposes, no repack).
  - Main matmul fully bf16: W0 shard [17024,1024] bf16 streamed through
    a deep SBUF ring (DMA saturated from t=0), 133 k-tiles x 2 matmuls.
  - AllReduce partial h over 8 cores, then replicated bf16 dense tail
    with resident W1/W2/Wout.
"""

import sys

sys.path.insert(0, "/opt/trn_rl_repo")

import numpy as np
import ml_dtypes

import concourse.bass as bass
import concourse.bacc as bacc
import concourse.mybir as mybir
from concourse import tile
from concourse import bass_utils


class _SkipRest(Exception):
    pass


F32 = mybir.dt.float32
F16 = mybir.dt.float16
BF16 = mybir.dt.bfloat16
I16 = mybir.dt.int16
U8 = mybir.dt.uint8
ALU = mybir.AluOpType
ACTF = mybir.ActivationFunctionType

B, M, V, H, TD = 32, 512, 256, 1024, 10
NC = 8
RPC = M // NC              # 64 ranks per core
MEMROWS = B * M            # 16384
MEMP = MEMROWS + B         # 16416 gather-source rows (memory + x)
NKT = 2 * RPC              # 128 memory k-tiles per core
NBT = RPC * TD // 128      # 5 bits k-tiles per core
W0S_ROWS = RPC * V + RPC * TD  # 17024
NIDX = RPC * B             # 2048 gather items
W0_RING = 52               # W0 SBUF prefetch ring depth (tiles of [128,1024] bf16)


def build_program(stage="full"):
    lvl = {"idx": 0, "gat": 1, "parth": 2, "full": 3}[stage]
    nc = bacc.Bacc(
        "TRN2",
        target_bir_lowering=False,
        debug=False,
        enable_asserts=False,
        num_devices=NC,
    )

    def din(name, shape, dtype=F32):
        return nc.dram_tensor(name, list(shape), dtype, kind="ExternalInput").ap()

    mem_plus = din("mem_plus", (MEMP, V), BF16)
    timings = din("timings", (B, M))
    msur = din("msur", (B, M))
    w0s = din("W0s", (W0S_ROWS, H), BF16)
    w1 = din("W1", (H, H), BF16)
    w2 = din("W2", (H, H), BF16)
    wout = din("Wout", (H, V), BF16)
    b0r = din("b0r", (B, H))
    b1r = din("b1r", (B, H))
    b2r = din("b2r", (B, H))
    boutr = din("boutr", (B, V))
    c_eye = din("c_eye", (128, 128))
    c_eye16 = din("c_eye16", (128, 128), BF16)
    c_esel = din("c_esel", (B, B * 128))
    c_iota = din("c_iota512", (B, M))
    c_gdata = din("c_gdata", (B, M))
    c_xrow = din("c_xrow", (B, M))
    c_roff = din("c_roff", (B, 1))

    out = nc.dram_tensor("out", [B, V], F32, kind="ExternalOutput").ap()
    dbg = (nc.dram_tensor("dbg", [128, 256], F32, kind="ExternalOutput").ap()
           if stage != "full" else None)

    with tile.TileContext(nc) as tc:
        with (
            tc.tile_pool(name="const", bufs=1) as constp,
            tc.tile_pool(name="state", bufs=1) as state,
            tc.tile_pool(name="w0t", bufs=W0_RING) as w0p,
            tc.tile_pool(name="pkrep", bufs=2, space="PSUM") as pkp,
            tc.tile_pool(name="ptr", bufs=2, space="PSUM") as ptrp,
            tc.tile_pool(name="ptrb", bufs=1, space="PSUM") as ptrbp,
            tc.tile_pool(name="ph", bufs=1, space="PSUM") as php,
            tc.tile_pool(name="dram", bufs=1, space="DRAM") as dramp,
        ):
            def load(pool, ap, dtype=None):
                t = pool.tile(list(ap.shape), dtype or ap.dtype,
                              tag=f"ld_{ap.tensor.name}")
                nc.sync.dma_start(t[:], ap)
                return t

            # ---- constants / state / resident weights ----------------
            eye = load(constp, c_eye)
            eye16 = load(constp, c_eye16)
            esel = load(constp, c_esel)
            iota = load(constp, c_iota)
            gdata = load(constp, c_gdata)
            xrow = load(constp, c_xrow)
            roff = load(constp, c_roff)
            b0s = load(constp, b0r)
            b1s = load(constp, b1r)
            b2s = load(constp, b2r)
            bouts = load(constp, boutr)
            t_sb = load(state, timings)
            ms_sb = load(state, msur)

            # ---- stage A: argmin slot + keys + scatter data ----------
            msur2 = state.tile([B, M], F32, tag="msur2")
            nc.vector.tensor_scalar(msur2[:], ms_sb[:], 0.9, None, ALU.mult)
            minv = state.tile([B, 1], F32, tag="minv")
            nc.vector.tensor_reduce(minv[:], msur2[:], axis=mybir.AxisListType.X,
                                    op=ALU.min)
            mask = state.tile([B, M], U8, tag="mask")
            nc.vector.tensor_scalar(mask[:], msur2[:], minv[:], None, ALU.is_equal)
            cand = state.tile([B, M], F32, tag="cand")
            nc.vector.memset(cand[:], 1.0e9)
            nc.vector.copy_predicated(cand[:], mask[:], iota[:])
            idx0 = state.tile([B, 1], F32, tag="idx0")
            nc.vector.tensor_reduce(idx0[:], cand[:], axis=mybir.AxisListType.X,
                                    op=ALU.min)

            keys = state.tile([B, M], F32, tag="keys")
            nc.vector.tensor_scalar(keys[:], t_sb[:], 512.0, 512.0, ALU.mult, ALU.add)
            nc.vector.tensor_tensor(keys[:], keys[:], iota[:], ALU.add)
            mask2 = state.tile([B, M], U8, tag="mask2")
            nc.vector.tensor_scalar(mask2[:], iota[:], idx0[:], None, ALU.is_equal)
            nc.vector.copy_predicated(keys[:], mask2[:], iota[:])

            # effective timings (t+1, overridden slot -> 0) for bits
            teff = state.tile([B, M], F32, tag="teff")
            nc.vector.tensor_scalar(teff[:], t_sb[:], 1.0, None, ALU.add)
            zeros = state.tile([B, M], F32, tag="zeros")
            nc.vector.memset(zeros[:], 0.0)
            nc.vector.copy_predicated(teff[:], mask2[:], zeros[:])
            st16 = state.tile([B, M], F16, tag="st16")
            nc.vector.tensor_copy(st16[:], teff[:])

            # gather data values (row ids), override slot -> x row
            gd = state.tile([B, M], F32, tag="gd")
            nc.vector.tensor_copy(gd[:], gdata[:])
            nc.vector.copy_predicated(gd[:], mask2[:], xrow[:])
            gd16 = state.tile([B, M], I16, tag="gd16")
            nc.vector.tensor_copy(gd16[:], gd[:])

            # ---- keysT via PE transpose ------------------------------
            keysT = state.tile([128, 4 * B], F32, tag="keysT")
            for mt in range(4):
                ptt = ptrp.tile([128, 128], F32, tag="pm")
                nc.tensor.transpose(ptt[:, 0:B], keys[:, mt * 128:(mt + 1) * 128],
                                    eye[0:B, 0:B])
                nc.scalar.activation(keysT[:, mt * B:(mt + 1) * B], ptt[:, 0:B],
                                     ACTF.Copy)

            # ---- ranks: brute force split DVE / Activation -----------
            # rank_sb[:, mt*B + b] = rank of slot (mt*128+p) for batch b
            rank_sb = state.tile([128, 4 * B], F32, tag="rank")
            scr_v = state.tile([128, M], BF16, tag="scr_v")
            scr_a = state.tile([128, M], BF16, tag="scr_a")
            for b in range(B):
                pk_t = pkp.tile([128, M], F32, tag="pkrep")
                nc.tensor.matmul(pk_t[:], esel[:, b * 128:(b + 1) * 128], keys[:],
                                 start=True, stop=True)
                for mt in range(4):
                    col = mt * B + b
                    sc = keysT[:, col:col + 1]
                    if mt < 2:
                        nc.vector.tensor_scalar(
                            scr_v[:], pk_t[:], sc, None, ALU.is_lt, ALU.add,
                            accum_out=rank_sb[:, col:col + 1])
                    else:
                        nc.scalar.activation(
                            scr_a[:], pk_t[:], ACTF.Sign, bias=sc, scale=-1.0,
                            accum_out=rank_sb[:, col:col + 1])
            # Sign cols: rank = (S + 511) / 2
            nc.vector.tensor_scalar(rank_sb[:, 2 * B:4 * B], rank_sb[:, 2 * B:4 * B],
                                    0.5, 255.5, ALU.mult, ALU.add)

            # ---- rank -> [B, M] layout via PE transpose --------------
            rankT = state.tile([B, M], F32, tag="rankT")
            for mt in range(4):
                ptt = ptrp.tile([128, 128], F32, tag="pm")
                nc.tensor.transpose(ptt[0:B, 0:128],
                                    rank_sb[:, mt * B:(mt + 1) * B], eye[:])
                nc.scalar.activation(rankT[:, mt * 128:(mt + 1) * 128],
                                     ptt[0:B, 0:128], ACTF.Copy)

            # window: idxs = (rank - 64c) if in [0,64) else negative
            ri = state.tile([B, M], F32, tag="ri")
            nc.vector.tensor_scalar(ri[:], rankT[:], roff[:], None, ALU.subtract)
            tmask = state.tile([B, M], F32, tag="tmask")
            nc.vector.tensor_scalar(tmask[:], ri[:], 64.0, None, ALU.is_lt)
            nc.vector.tensor_tensor(ri[:], ri[:], tmask[:], ALU.mult)
            nc.vector.tensor_tensor(ri[:], ri[:], tmask[:], ALU.add)
            sidx = state.tile([B, M], I16, tag="sidx")
            nc.vector.tensor_scalar(sidx[:], ri[:], 1.0, None, ALU.subtract)

            # ---- local_scatter: invert permutation -------------------
            idxc = state.tile([B, RPC], I16, tag="idxc")
            nc.gpsimd.local_scatter(idxc[:], gd16[:], sidx[:],
                                    channels=B, num_elems=RPC, num_idxs=M)
            stc16 = state.tile([B, RPC], F16, tag="stc16")
            nc.gpsimd.local_scatter(stc16[:], st16[:], sidx[:],
                                    channels=B, num_elems=RPC, num_idxs=M)

            # ---- gather idx tile: [16, 64, 2] wrap + replicate -------
            idxfull = state.tile([128, RPC, 2], I16, tag="idxfull")
            nc.sync.dma_start(idxfull[0:16, :, 0], idxc[0:16, :])
            nc.sync.dma_start(idxfull[0:16, :, 1], idxc[16:32, :])
            idxflat = idxfull.rearrange("p r two -> p (r two)")
            for g in range(1, 8):
                nc.sync.dma_start(idxflat[16 * g:16 * (g + 1), :], idxflat[0:16, :])

            if stage == "idx":
                tmpd = state.tile([128, 128], F32, tag="tmpd")
                nc.vector.tensor_copy(tmpd[:], idxflat[:])
                nc.sync.dma_start(dbg[:, 0:128], tmpd[:])
                tmpd2 = state.tile([B, RPC], F32, tag="tmpd2")
                nc.vector.tensor_copy(tmpd2[:], stc16[:])
                nc.sync.dma_start(dbg[0:B, 128:128 + RPC], tmpd2[:])
            try:
                if lvl < 1:
                    raise _SkipRest

                # ---- transposed gather: all 128 lhsT tiles -----------
                G = state.tile([128, 2, NIDX], BF16, tag="G")
                nc.gpsimd.dma_gather(
                    out_ap=G[:],
                    in_ap=mem_plus,
                    idxs_ap=idxflat[:],
                    num_idxs=NIDX,
                    num_idxs_reg=NIDX,
                    elem_size=V,
                    transpose=True,
                    single_packet=False,
                )

                # ---- bits for core's 64 ranks ------------------------
                stc = state.tile([B, RPC], F32, tag="stc")
                nc.vector.tensor_copy(stc[:], stc16[:])
                u_all = state.tile([B, RPC, TD], F32, tag="u_all")
                tmpu = state.tile([B, RPC], F32, tag="tmpu")
                rem = stc
                for j in range(9, -1, -1):
                    ud = u_all[:, :, j]
                    nc.vector.tensor_scalar(ud, rem[:], float(2 ** j), None, ALU.is_ge)
                    nc.vector.tensor_scalar(tmpu[:], ud, float(2 ** j), None, ALU.mult)
                    nc.vector.tensor_tensor(rem[:], rem[:], tmpu[:], ALU.subtract)
                u16 = state.tile([B, RPC * TD], BF16, tag="u16")
                nc.vector.tensor_copy(u16[:], u_all.rearrange("p r d -> p (r d)")[:])
                bitsT = state.tile([128, NBT * B], BF16, tag="bitsT")
                for tt in range(NBT):
                    ptt = ptrbp.tile([128, 256], BF16, tag="pmb")
                    nc.tensor.transpose(ptt[:, 0:B],
                                        u16[:, tt * 128:(tt + 1) * 128],
                                        eye16[0:B, 0:B])
                    nc.scalar.activation(bitsT[:, tt * B:(tt + 1) * B],
                                         ptt[:, 0:B], ACTF.Copy)

                if stage == "gat":
                    tmpg = state.tile([128, 256], F32, tag="tmpg")
                    nc.vector.tensor_copy(tmpg[:, 0:128], G[:, 0, 0:128])
                    nc.vector.tensor_copy(tmpg[:, 128:160], bitsT[:, 0:32])
                    nc.vector.tensor_copy(tmpg[:, 160:256], G[:, 1, 0:96])
                    nc.sync.dma_start(dbg[:, :], tmpg[:])
                if lvl < 2:
                    raise _SkipRest

                # ---- main matmul: partial h = pred_in_shard @ W0_shard
                nkt_all = NKT + NBT
                ph_t = php.tile([B, H], F32, tag="ph")
                for kt in range(nkt_all):
                    w0t = w0p.tile([128, H], BF16, tag="w0t")
                    nc.sync.dma_start(w0t[:], w0s[kt * 128:(kt + 1) * 128, :])
                    if kt < NKT:
                        r, hh = kt // 2, kt % 2
                        lhsT = G[:, hh, B * r:B * (r + 1)]
                    else:
                        tb = kt - NKT
                        lhsT = bitsT[:, tb * B:(tb + 1) * B]
                    last = kt == nkt_all - 1
                    nc.tensor.matmul(ph_t[:, 0:512], lhsT, w0t[:, 0:512],
                                     start=(kt == 0), stop=last)
                    nc.tensor.matmul(ph_t[:, 512:1024], lhsT, w0t[:, 512:1024],
                                     start=(kt == 0), stop=last)

                part_h = state.tile([B, H], F32, tag="part_h")
                nc.vector.tensor_copy(part_h[:], ph_t[:])
                if stage == "parth":
                    nc.sync.dma_start(dbg[0:B, 0:256], part_h[:, 0:256])
                if lvl < 3:
                    raise _SkipRest

                # ---- AllReduce partial h over the 8 cores ------------
                cc_in = dramp.tile([B, H], F32, tag="cc_in")
                cc_out = dramp.tile([B, H], F32, tag="cc_out")
                nc.sync.dma_start(cc_in[:], part_h[:])
                nc.gpsimd.collective_compute(
                    "AllReduce", ALU.add,
                    replica_groups=[list(range(NC))],
                    ins=[cc_in.opt()],
                    outs=[cc_out.opt()],
                )
                h_sb = state.tile([B, H], F32, tag="h_sb")
                nc.sync.dma_start(h_sb[:], cc_out[:])

                # ---- dense tail (replicated, bf16 weights) -----------
                nc.vector.tensor_tensor(h_sb[:], h_sb[:], b0s[:], ALU.add)
                nc.vector.tensor_scalar(h_sb[:], h_sb[:], 0.0, None, ALU.max)

                def dense(h_in, w_dram, bias_sb, n_out, relu, tag):
                    hT = state.tile([128, 8 * B], BF16, tag=f"hT_{tag}")
                    for kt in range(8):
                        ptt = ptrp.tile([128, 128], F32, tag="pm")
                        nc.tensor.transpose(ptt[:, 0:B],
                                            h_in[:, kt * 128:(kt + 1) * 128],
                                            eye[0:B, 0:B])
                        nc.scalar.activation(hT[:, kt * B:(kt + 1) * B],
                                             ptt[:, 0:B], ACTF.Copy)
                    pho = php.tile([B, n_out], F32, tag="ph")
                    for kt in range(8):
                        wt = w0p.tile([128, H], BF16, tag="w0t")
                        nc.sync.dma_start(wt[:, 0:n_out],
                                          w_dram[kt * 128:(kt + 1) * 128, :])
                        for j0 in range(0, n_out, 512):
                            jn = min(512, n_out - j0)
                            nc.tensor.matmul(
                                pho[:, j0:j0 + jn], hT[:, kt * B:(kt + 1) * B],
                                wt[:, j0:j0 + jn],
                                start=(kt == 0), stop=(kt == 7))
                    h_next = state.tile([B, n_out], F32, tag=f"h_{tag}")
                    nc.vector.tensor_tensor(h_next[:], pho[:], bias_sb[:], ALU.add)
                    if relu:
                        nc.vector.tensor_scalar(h_next[:], h_next[:], 0.0, None,
                                                ALU.max)
                    return h_next

                h1 = dense(h_sb, w1, b1s, H, True, "l1")
                h2 = dense(h1, w2, b2s, H, True, "l2")
                logits = dense(h2, wout, bouts, V, False, "lo")
                nc.sync.dma_start(out, logits[:])
            except _SkipRest:
                pass

    nc.compile()
    return nc


def _esel():
    esel = np.zeros((B, B * 128), np.float32)
    for b in range(B):
        esel[b, b * 128:(b + 1) * 128] = 1.0
    return esel


def make_in_maps(inputs):
    x = np.asarray(inputs["x"], np.float32)
    memory = np.asarray(inputs["memory"], np.float32)
    timings = np.asarray(inputs["memory_timings"], np.float32)
    msur = np.asarray(inputs["memory_surprise"], np.float32)
    W0 = np.asarray(inputs["W0"], np.float32)
    W1 = np.asarray(inputs["W1"], np.float32)
    W2 = np.asarray(inputs["W2"], np.float32)
    Wout = np.asarray(inputs["Wout"], np.float32)
    b0 = np.asarray(inputs["b0"], np.float32)
    b1 = np.asarray(inputs["b1"], np.float32)
    b2 = np.asarray(inputs["b2"], np.float32)
    bout = np.asarray(inputs["bout"], np.float32)

    bf = ml_dtypes.bfloat16
    mem_plus = np.concatenate([memory.reshape(MEMROWS, V), x], 0).astype(bf)
    W0b = W0.astype(bf)
    W1b = np.ascontiguousarray(W1.astype(bf))
    W2b = np.ascontiguousarray(W2.astype(bf))
    Woutb = np.ascontiguousarray(Wout.astype(bf))

    iota512 = np.broadcast_to(np.arange(M, dtype=np.float32), (B, M)).copy()
    gdata = (512.0 * np.arange(B, dtype=np.float32)[:, None] + iota512)
    xrowc = np.broadcast_to(
        (MEMROWS + np.arange(B, dtype=np.float32))[:, None], (B, M)).copy()

    shared = {
        "mem_plus": mem_plus,
        "timings": timings,
        "msur": msur,
        "W1": W1b, "W2": W2b, "Wout": Woutb,
        "b0r": np.broadcast_to(b0, (B, H)).copy(),
        "b1r": np.broadcast_to(b1, (B, H)).copy(),
        "b2r": np.broadcast_to(b2, (B, H)).copy(),
        "boutr": np.broadcast_to(bout, (B, V)).copy(),
        "c_eye": np.eye(128, dtype=np.float32),
        "c_eye16": np.eye(128, dtype=np.float32).astype(bf),
        "c_esel": _esel(),
        "c_iota512": iota512,
        "c_gdata": gdata,
        "c_xrow": xrowc,
    }

    in_maps = []
    for core in range(NC):
        w0shard = np.concatenate(
            [W0b[core * RPC * V:(core + 1) * RPC * V],
             W0b[M * V + core * RPC * TD: M * V + (core + 1) * RPC * TD]], 0)
        m = dict(shared)
        m["W0s"] = np.ascontiguousarray(w0shard)
        m["c_roff"] = np.full((B, 1), 64.0 * core, np.float32)
        in_maps.append(m)
    return in_maps


_NC_CACHE = None


def kernel(**inputs) -> np.ndarray:
    global _NC_CACHE
    if _NC_CACHE is None:
        _NC_CACHE = build_program()
    nc = _NC_CACHE
    in_maps = make_in_maps(inputs)
    res = bass_utils.run_bass_kernel_spmd(nc, in_maps, core_ids=list(range(NC)))
    return np.asarray(res.results[0]["out"], np.float32)


if __name__ == "__main__":
    np.random.seed(0)
    build_program()
    print("build OK")


# revision 20
# speedup vs baseline: 2.0072x; 1.0893x over previous
"""Trainium2 Bass kernel for nn_Net_71270687310327 (scatter_memory).

Computation (see reference): argmin-scatter into memory, stable sort by
timings, gather sorted rows + timing bits, 4-layer MLP.

Design (v2):
  - keys = (t+1)*512 + m, argmin slot overridden to key=m (rank 0).
  - ranks by brute-force count, split across DVE (is_lt+accum) and the
    Activation engine (Sign trick: rank = (sum sign(k - k') + 511)/2),
    reading the per-batch broadcast keys (krep) directly from PSUM.
  - gpsimd local_scatter inverts the permutation: scatter gather-row ids
    (512b+m, x-row override baked in) and effective timings to positions
    rank-64c (out-of-window ranks -> negative idx, dropped).
  - the [16-lane-wrapped, replicated] gather index tile is built with two
    accumulated PE selector matmuls (no serialized SBUF DMA chain).
  - two gpsimd dma_gather(transpose=True) calls on bf16 mem_plus yield
    all 128 transposed lhsT k-tiles directly.
  - W0 shard bf16 is streamed in 8-tile chunk DMAs (few SP issues),
    issued at the top of the program so the stream saturates from t=0.
  - AllReduce partial h, then replicated bf16 dense tail (chunk DMAs).
"""

import sys

sys.path.insert(0, "/opt/trn_rl_repo")

import numpy as np
import ml_dtypes

import concourse.bass as bass
import concourse.bacc as bacc
import concourse.mybir as mybir
from concourse import tile
from concourse import bass_utils


class _SkipRest(Exception):
    pass


F32 = mybir.dt.float32
F16 = mybir.dt.float16
BF16 = mybir.dt.bfloat16
I16 = mybir.dt.int16
U8 = mybir.dt.uint8
ALU = mybir.AluOpType
ACTF = mybir.ActivationFunctionType

B, M, V, H, TD = 32, 512, 256, 1024, 10
NC = 8
RPC = M // NC              # 64 ranks per core
MEMROWS = B * M            # 16384
MEMP = MEMROWS + B         # 16416 gather-source rows (memory + x)
NKT = 2 * RPC              # 128 memory k-tiles per core
NBT = RPC * TD // 128      # 5 bits k-tiles per core
NTILES = NKT + NBT         # 133 live k-tiles
CHUNK = 8                  # k-tiles per W0 DMA
NCHUNK = 17                # ceil(133/8) -> padded to 136 tiles
W0S_ROWS = NCHUNK * CHUNK * 128  # 17408 (incl. 3 zero-pad tiles)
NIDX = RPC * B             # 2048 gather items
W0_RING = 6                # chunk ring depth (6 x 16KB/partition)

# c32 constant-pack column offsets (f32, 32 partitions)
_offs = {}
_w = 0
for _name, _n in [("esel", B * 128), ("iota", M), ("gdata", M), ("xrow", M),
                  ("b0r", H), ("b1r", H), ("b2r", H), ("boutr", V),
                  ("timings", M), ("msur", M), ("roff", 4),
                  ("E0", 128), ("E1", 128)]:
    _offs[_name] = _w
    _w += _n
C32W = _w


def build_program(stage="full"):
    lvl = {"idx": 0, "gat": 1, "parth": 2, "full": 3}[stage]
    nc = bacc.Bacc(
        "TRN2",
        target_bir_lowering=False,
        debug=False,
        enable_asserts=False,
        num_devices=NC,
    )

    def din(name, shape, dtype=F32):
        return nc.dram_tensor(name, list(shape), dtype, kind="ExternalInput").ap()

    mem_plus = din("mem_plus", (MEMP, V), BF16)
    w0s = din("W0s", (W0S_ROWS, H), BF16)
    w1 = din("W1", (H, H), BF16)
    w2 = din("W2", (H, H), BF16)
    wout = din("Wout", (H, V), BF16)
    c32 = din("c32", (B, C32W))
    c_eye = din("c_eye", (128, 128))
    c_eye16 = din("c_eye16", (128, 128), BF16)

    out = nc.dram_tensor("out", [B, V], F32, kind="ExternalOutput").ap()
    dbg = (nc.dram_tensor("dbg", [128, 256], F32, kind="ExternalOutput").ap()
           if stage != "full" else None)

    with tile.TileContext(nc) as tc:
        with (
            tc.tile_pool(name="const", bufs=1) as constp,
            tc.tile_pool(name="state", bufs=1) as state,
            tc.tile_pool(name="w0c", bufs=W0_RING) as w0p,
            tc.tile_pool(name="pkrep", bufs=3, space="PSUM") as pkp,
            tc.tile_pool(name="ptr", bufs=2, space="PSUM") as ptrp,
            tc.tile_pool(name="ptrb", bufs=1, space="PSUM") as ptrbp,
            tc.tile_pool(name="ph", bufs=1, space="PSUM") as php,
            tc.tile_pool(name="dram", bufs=1, space="DRAM") as dramp,
        ):
            # ---- 3 const DMAs, then the W0 chunk stream (all on SP) --
            cpack = constp.tile([B, C32W], F32, tag="cpack")
            nc.sync.dma_start(cpack[:], c32)
            eye = constp.tile([128, 128], F32, tag="eye")
            nc.sync.dma_start(eye[:], c_eye)
            eye16 = constp.tile([128, 128], BF16, tag="eye16")
            nc.sync.dma_start(eye16[:], c_eye16)

            def cc(name, n):
                o = _offs[name]
                return cpack[:, o:o + n]

            esel = cc("esel", B * 128)
            iota = cc("iota", M)
            gdata = cc("gdata", M)
            xrow = cc("xrow", M)
            b0s = cc("b0r", H)
            b1s = cc("b1r", H)
            b2s = cc("b2r", H)
            bouts = cc("boutr", V)
            t_sb = cc("timings", M)
            ms_sb = cc("msur", M)
            roff = cc("roff", 1)
            E0 = cc("E0", 128)
            E1 = cc("E1", 128)

            w0view = w0s.rearrange("(kc j p) c -> kc p j c", p=128, j=CHUNK)
            w0tiles = []
            for kc in range(NCHUNK):
                w0t = w0p.tile([128, CHUNK, H], BF16, tag="w0c")
                nc.sync.dma_start(w0t[:], w0view[kc])
                w0tiles.append(w0t)

            # ---- stage A: argmin slot + keys + scatter data ----------
            msur2 = state.tile([B, M], F32, tag="msur2")
            nc.vector.tensor_scalar(msur2[:], ms_sb, 0.9, None, ALU.mult)
            minv = state.tile([B, 1], F32, tag="minv")
            nc.vector.tensor_reduce(minv[:], msur2[:], axis=mybir.AxisListType.X,
                                    op=ALU.min)
            mask = state.tile([B, M], U8, tag="mask")
            nc.vector.tensor_scalar(mask[:], msur2[:], minv[:], None, ALU.is_equal)
            cand = state.tile([B, M], F32, tag="cand")
            nc.vector.memset(cand[:], 1.0e9)
            nc.vector.copy_predicated(cand[:], mask[:], iota)
            idx0 = state.tile([B, 1], F32, tag="idx0")
            nc.vector.tensor_reduce(idx0[:], cand[:], axis=mybir.AxisListType.X,
                                    op=ALU.min)

            keys = state.tile([B, M], F32, tag="keys")
            nc.vector.tensor_scalar(keys[:], t_sb, 512.0, 512.0, ALU.mult, ALU.add)
            nc.vector.tensor_tensor(keys[:], keys[:], iota, ALU.add)
            mask2 = state.tile([B, M], U8, tag="mask2")
            nc.vector.tensor_scalar(mask2[:], iota, idx0[:], None, ALU.is_equal)
            nc.vector.copy_predicated(keys[:], mask2[:], iota)

            # effective timings (t+1, overridden slot -> 0) for bits
            teff = state.tile([B, M], F32, tag="teff")
            nc.scalar.activation(teff[:], t_sb, ACTF.Copy, bias=1.0)
            zeros = state.tile([B, M], F32, tag="zeros")
            nc.vector.memset(zeros[:], 0.0)
            nc.vector.copy_predicated(teff[:], mask2[:], zeros[:])
            st16 = state.tile([B, M], F16, tag="st16")
            nc.vector.tensor_copy(st16[:], teff[:])

            # gather data values (row ids), override slot -> x row
            gd = state.tile([B, M], F32, tag="gd")
            nc.scalar.activation(gd[:], gdata, ACTF.Copy)
            nc.vector.copy_predicated(gd[:], mask2[:], xrow)
            gd16 = state.tile([B, M], I16, tag="gd16")
            nc.vector.tensor_copy(gd16[:], gd[:])

            # ---- keysT via PE transpose ------------------------------
            keysT = state.tile([128, 4 * B], F32, tag="keysT")
            for mt in range(4):
                ptt = ptrp.tile([128, 128], F32, tag="pm")
                nc.tensor.transpose(ptt[:, 0:B], keys[:, mt * 128:(mt + 1) * 128],
                                    eye[0:B, 0:B])
                nc.scalar.activation(keysT[:, mt * B:(mt + 1) * B], ptt[:, 0:B],
                                     ACTF.Copy)

            # ---- ranks: brute force split DVE / Activation -----------
            rank_sb = state.tile([128, 4 * B], F32, tag="rank")
            scr_v = state.tile([128, M], BF16, tag="scr_v")
            scr_a = state.tile([128, M], BF16, tag="scr_a")
            for b in range(B):
                pk_t = pkp.tile([128, M], F32, tag="pkrep")
                nc.tensor.matmul(pk_t[:], esel[:, b * 128:(b + 1) * 128], keys[:],
                                 start=True, stop=True)
                for mt in range(4):
                    col = mt * B + b
                    sc = keysT[:, col:col + 1]
                    if mt < 2:
                        nc.vector.tensor_scalar(
                            scr_v[:], pk_t[:], sc, None, ALU.is_lt, ALU.add,
                            accum_out=rank_sb[:, col:col + 1])
                    else:
                        nc.scalar.activation(
                            scr_a[:], pk_t[:], ACTF.Sign, bias=sc, scale=-1.0,
                            accum_out=rank_sb[:, col:col + 1])
            nc.vector.tensor_scalar(rank_sb[:, 2 * B:4 * B], rank_sb[:, 2 * B:4 * B],
                                    0.5, 255.5, ALU.mult, ALU.add)

            # ---- rank -> [B, M] layout via PE transpose --------------
            rankT = state.tile([B, M], F32, tag="rankT")
            for mt in range(4):
                ptt = ptrp.tile([128, 128], F32, tag="pm")
                nc.tensor.transpose(ptt[0:B, 0:128],
                                    rank_sb[:, mt * B:(mt + 1) * B], eye[:])
                nc.scalar.activation(rankT[:, mt * 128:(mt + 1) * 128],
                                     ptt[0:B, 0:128], ACTF.Copy)

            # window: idxs = (rank - 64c) if in [0,64) else negative
            ri = state.tile([B, M], F32, tag="ri")
            nc.vector.tensor_scalar(ri[:], rankT[:], roff, None, ALU.subtract)
            tmask = state.tile([B, M], F32, tag="tmask")
            nc.vector.tensor_scalar(tmask[:], ri[:], 64.0, None, ALU.is_lt)
            nc.vector.tensor_tensor(ri[:], ri[:], tmask[:], ALU.mult)
            nc.vector.tensor_tensor(ri[:], ri[:], tmask[:], ALU.add)
            sidx = state.tile([B, M], I16, tag="sidx")
            nc.vector.tensor_scalar(sidx[:], ri[:], 1.0, None, ALU.subtract)

            # ---- local_scatter: invert permutation -------------------
            idxc = state.tile([B, RPC], I16, tag="idxc")
            nc.gpsimd.local_scatter(idxc[:], gd16[:], sidx[:],
                                    channels=B, num_elems=RPC, num_idxs=M)
            stc16 = state.tile([B, RPC], F16, tag="stc16")
            nc.gpsimd.local_scatter(stc16[:], st16[:], sidx[:],
                                    channels=B, num_elems=RPC, num_idxs=M)

            # ---- gather idx tile via PE selector broadcast -----------
            # idxfull[p, f] = idxc[p%16 + 16*(f%2), f//2], replicated over
            # the 8 gpsimd cores (p//16).
            idxf = state.tile([B, RPC], F32, tag="idxf")
            nc.vector.tensor_copy(idxf[:], idxc[:])
            Y = state.tile([B, 2, 128], F32, tag="Y")
            nc.vector.memset(Y[:], 0.0)
            yv = Y.rearrange("p two (r par) -> p two r par", par=2)
            nc.vector.tensor_copy(yv[:, 0, :, 0], idxf[:])
            nc.vector.tensor_copy(yv[:, 1, :, 1], idxf[:])
            pidx = ptrp.tile([128, 128], F32, tag="pm")
            nc.tensor.matmul(pidx[:], E0, Y[:, 0, :], start=True, stop=False)
            nc.tensor.matmul(pidx[:], E1, Y[:, 1, :], start=False, stop=True)
            idxfull = state.tile([128, 128], I16, tag="idxfull")
            nc.vector.tensor_copy(idxfull[:], pidx[:])

            if stage == "idx":
                tmpd = state.tile([128, 128], F32, tag="tmpd")
                nc.vector.tensor_copy(tmpd[:], idxfull[:])
                nc.scalar.dma_start(dbg[:, 0:128], tmpd[:])
                tmpd2 = state.tile([B, RPC], F32, tag="tmpd2")
                nc.vector.tensor_copy(tmpd2[:], stc16[:])
                nc.scalar.dma_start(dbg[0:B, 128:128 + RPC], tmpd2[:])
            try:
                if lvl < 1:
                    raise _SkipRest

                # ---- transposed gathers: all 128 lhsT tiles ----------
                G0 = state.tile([128, 2, NIDX // 2], BF16, tag="G0")
                G1 = state.tile([128, 2, NIDX // 2], BF16, tag="G1")
                nc.gpsimd.dma_gather(
                    out_ap=G0[:], in_ap=mem_plus, idxs_ap=idxfull[:, 0:64],
                    num_idxs=NIDX // 2, num_idxs_reg=NIDX // 2,
                    elem_size=V, transpose=True, single_packet=False)
                nc.gpsimd.dma_gather(
                    out_ap=G1[:], in_ap=mem_plus, idxs_ap=idxfull[:, 64:128],
                    num_idxs=NIDX // 2, num_idxs_reg=NIDX // 2,
                    elem_size=V, transpose=True, single_packet=False)

                # ---- bits for core's 64 ranks ------------------------
                stc = state.tile([B, RPC], F32, tag="stc")
                nc.vector.tensor_copy(stc[:], stc16[:])
                u_all = state.tile([B, RPC, TD], F32, tag="u_all")
                tmpu = state.tile([B, RPC], F32, tag="tmpu")
                rem = stc
                for j in range(9, -1, -1):
                    ud = u_all[:, :, j]
                    nc.vector.tensor_scalar(ud, rem[:], float(2 ** j), None, ALU.is_ge)
                    nc.vector.tensor_scalar(tmpu[:], ud, float(2 ** j), None, ALU.mult)
                    nc.vector.tensor_tensor(rem[:], rem[:], tmpu[:], ALU.subtract)
                u16 = state.tile([B, RPC * TD], BF16, tag="u16")
                nc.vector.tensor_copy(u16[:], u_all.rearrange("p r d -> p (r d)")[:])
                bitsT = state.tile([128, NBT * B], BF16, tag="bitsT")
                for tt in range(NBT):
                    ptb = ptrbp.tile([128, 256], BF16, tag="pmb")
                    nc.tensor.transpose(ptb[:, 0:B],
                                        u16[:, tt * 128:(tt + 1) * 128],
                                        eye16[0:B, 0:B])
                    nc.scalar.activation(bitsT[:, tt * B:(tt + 1) * B],
                                         ptb[:, 0:B], ACTF.Copy)

                if stage == "gat":
                    tmpg = state.tile([128, 256], F32, tag="tmpg")
                    nc.vector.tensor_copy(tmpg[:, 0:128], G0[:, 0, 0:128])
                    nc.vector.tensor_copy(tmpg[:, 128:160], bitsT[:, 0:32])
                    nc.vector.tensor_copy(tmpg[:, 160:256], G1[:, 1, 0:96])
                    nc.scalar.dma_start(dbg[:, :], tmpg[:])
                if lvl < 2:
                    raise _SkipRest

                # ---- main matmul: partial h = pred_in_shard @ W0_shard
                ph_t = php.tile([B, H], F32, tag="ph")
                for kt in range(NTILES):
                    w0t = w0tiles[kt // CHUNK]
                    j = kt % CHUNK
                    if kt < NKT:
                        r, hh = kt // 2, kt % 2
                        if r < 32:
                            lhsT = G0[:, hh, B * r:B * (r + 1)]
                        else:
                            lhsT = G1[:, hh, B * (r - 32):B * (r - 31)]
                    else:
                        tb = kt - NKT
                        lhsT = bitsT[:, tb * B:(tb + 1) * B]
                    last = kt == NTILES - 1
                    nc.tensor.matmul(ph_t[:, 0:512], lhsT, w0t[:, j, 0:512],
                                     start=(kt == 0), stop=last)
                    nc.tensor.matmul(ph_t[:, 512:1024], lhsT, w0t[:, j, 512:1024],
                                     start=(kt == 0), stop=last)

                part_h = state.tile([B, H], F32, tag="part_h")
                nc.vector.tensor_copy(part_h[:], ph_t[:])
                if stage == "parth":
                    nc.scalar.dma_start(dbg[0:B, 0:256], part_h[:, 0:256])
                if lvl < 3:
                    raise _SkipRest

                # ---- AllReduce partial h over the 8 cores ------------
                cc_in = dramp.tile([B, H], F32, tag="cc_in")
                cc_out = dramp.tile([B, H], F32, tag="cc_out")
                nc.sync.dma_start(cc_in[:], part_h[:])
                nc.gpsimd.collective_compute(
                    "AllReduce", ALU.add,
                    replica_groups=[list(range(NC))],
                    ins=[cc_in.opt()],
                    outs=[cc_out.opt()],
                )
                h_sb = state.tile([B, H], F32, tag="h_sb")
                nc.sync.dma_start(h_sb[:], cc_out[:])

                # ---- dense tail (replicated, bf16 weights) -----------
                nc.vector.tensor_tensor(h_sb[:], h_sb[:], b0s, ALU.add)
                nc.vector.tensor_scalar(h_sb[:], h_sb[:], 0.0, None, ALU.max)

                def dense(h_in, w_dram, bias_sb, n_out, relu, tag):
                    wt = w0p.tile([128, CHUNK, H], BF16, tag="w0c")
                    wv = w_dram.rearrange("(j p) c -> p j c", p=128, j=CHUNK)
                    nc.sync.dma_start(wt[:, :, 0:n_out], wv)
                    hT = state.tile([128, 8 * B], BF16, tag=f"hT_{tag}")
                    for kt in range(8):
                        ptt = ptrp.tile([128, 128], F32, tag="pm")
                        nc.tensor.transpose(ptt[:, 0:B],
                                            h_in[:, kt * 128:(kt + 1) * 128],
                                            eye[0:B, 0:B])
                        nc.scalar.activation(hT[:, kt * B:(kt + 1) * B],
                                             ptt[:, 0:B], ACTF.Copy)
                    pho = php.tile([B, n_out], F32, tag="ph")
                    for kt in range(8):
                        for j0 in range(0, n_out, 512):
                            jn = min(512, n_out - j0)
                            nc.tensor.matmul(
                                pho[:, j0:j0 + jn], hT[:, kt * B:(kt + 1) * B],
                                wt[:, kt, j0:j0 + jn],
                                start=(kt == 0), stop=(kt == 7))
                    h_next = state.tile([B, n_out], F32, tag=f"h_{tag}")
                    nc.vector.tensor_tensor(h_next[:], pho[:], bias_sb, ALU.add)
                    if relu:
                        nc.vector.tensor_scalar(h_next[:], h_next[:], 0.0, None,
                                                ALU.max)
                    return h_next

                h1 = dense(h_sb, w1, b1s, H, True, "l1")
                h2 = dense(h1, w2, b2s, H, True, "l2")
                logits = dense(h2, wout, bouts, V, False, "lo")
                nc.sync.dma_start(out, logits[:])
            except _SkipRest:
                pass

    nc.compile()
    return nc


def make_in_maps(inputs):
    x = np.asarray(inputs["x"], np.float32)
    memory = np.asarray(inputs["memory"], np.float32)
    timings = np.asarray(inputs["memory_timings"], np.float32)
    msur = np.asarray(inputs["memory_surprise"], np.float32)
    W0 = np.asarray(inputs["W0"], np.float32)
    W1 = np.asarray(inputs["W1"], np.float32)
    W2 = np.asarray(inputs["W2"], np.float32)
    Wout = np.asarray(inputs["Wout"], np.float32)
    b0 = np.asarray(inputs["b0"], np.float32)
    b1 = np.asarray(inputs["b1"], np.float32)
    b2 = np.asarray(inputs["b2"], np.float32)
    bout = np.asarray(inputs["bout"], np.float32)

    bf = ml_dtypes.bfloat16
    mem_plus = np.concatenate([memory.reshape(MEMROWS, V), x], 0).astype(bf)
    W0b = W0.astype(bf)
    W1b = np.ascontiguousarray(W1.astype(bf))
    W2b = np.ascontiguousarray(W2.astype(bf))
    Woutb = np.ascontiguousarray(Wout.astype(bf))

    iota512 = np.broadcast_to(np.arange(M, dtype=np.float32), (B, M))
    gdata = 512.0 * np.arange(B, dtype=np.float32)[:, None] + iota512
    xrowc = np.broadcast_to(
        (MEMROWS + np.arange(B, dtype=np.float32))[:, None], (B, M))
    esel = np.zeros((B, B * 128), np.float32)
    for b in range(B):
        esel[b, b * 128:(b + 1) * 128] = 1.0
    p = np.arange(128)
    E0 = (np.arange(B)[:, None] == (p % 16)[None, :]).astype(np.float32)
    E1 = (np.arange(B)[:, None] == (p % 16 + 16)[None, :]).astype(np.float32)

    def pack(core):
        c = np.zeros((B, C32W), np.float32)

        def put(name, arr):
            o = _offs[name]
            c[:, o:o + arr.shape[1]] = arr

        put("esel", esel)
        put("iota", iota512)
        put("gdata", gdata)
        put("xrow", xrowc)
        put("b0r", np.broadcast_to(b0, (B, H)))
        put("b1r", np.broadcast_to(b1, (B, H)))
        put("b2r", np.broadcast_to(b2, (B, H)))
        put("boutr", np.broadcast_to(bout, (B, V)))
        put("timings", timings)
        put("msur", msur)
        put("roff", np.full((B, 4), 64.0 * core, np.float32))
        put("E0", E0)
        put("E1", E1)
        return c

    shared = {
        "mem_plus": mem_plus,
        "W1": W1b, "W2": W2b, "Wout": Woutb,
        "c_eye": np.eye(128, dtype=np.float32),
        "c_eye16": np.eye(128, dtype=np.float32).astype(bf),
    }

    in_maps = []
    for core in range(NC):
        w0shard = np.zeros((W0S_ROWS, H), bf)
        w0shard[:RPC * V] = W0b[core * RPC * V:(core + 1) * RPC * V]
        w0shard[RPC * V:RPC * V + RPC * TD] = (
            W0b[M * V + core * RPC * TD: M * V + (core + 1) * RPC * TD])
        m = dict(shared)
        m["W0s"] = w0shard
        m["c32"] = pack(core)
        in_maps.append(m)
    return in_maps


_NC_CACHE = None


def kernel(**inputs) -> np.ndarray:
    global _NC_CACHE
    if _NC_CACHE is None:
        _NC_CACHE = build_program()
    nc = _NC_CACHE
    in_maps = make_in_maps(inputs)
    res = bass_utils.run_bass_kernel_spmd(nc, in_maps, core_ids=list(range(NC)))
    return np.asarray(res.results[0]["out"], np.float32)


if __name__ == "__main__":
    np.random.seed(0)
    build_program()
    print("build OK")


# revision 38
# speedup vs baseline: 2.2134x; 1.1028x over previous
"""Trainium2 Bass kernel for nn_Net_71270687310327 (scatter_memory).

Computation (see reference): argmin-scatter into memory, stable sort by
timings, gather sorted rows + timing bits, 4-layer MLP.

Design (v2):
  - keys = (t+1)*512 + m, argmin slot overridden to key=m (rank 0).
  - ranks by brute-force count, split across DVE (is_lt+accum) and the
    Activation engine (Sign trick: rank = (sum sign(k - k') + 511)/2),
    reading the per-batch broadcast keys (krep) directly from PSUM.
  - gpsimd local_scatter inverts the permutation: scatter gather-row ids
    (512b+m, x-row override baked in) and effective timings to positions
    rank-64c (out-of-window ranks -> negative idx, dropped).
  - the [16-lane-wrapped, replicated] gather index tile is built with two
    accumulated PE selector matmuls (no serialized SBUF DMA chain).
  - two gpsimd dma_gather(transpose=True) calls on bf16 mem_plus yield
    all 128 transposed lhsT k-tiles directly.
  - W0 shard bf16 is streamed in 8-tile chunk DMAs (few SP issues),
    issued at the top of the program so the stream saturates from t=0.
  - AllReduce partial h, then replicated bf16 dense tail (chunk DMAs).
"""

import sys

sys.path.insert(0, "/opt/trn_rl_repo")

import numpy as np
import ml_dtypes

import concourse.bass as bass
import concourse.bacc as bacc
import concourse.mybir as mybir
from concourse import tile
from concourse import bass_utils


class _SkipRest(Exception):
    pass


F32 = mybir.dt.float32
F16 = mybir.dt.float16
BF16 = mybir.dt.bfloat16
F8 = mybir.dt.float8e4
I16 = mybir.dt.int16
U8 = mybir.dt.uint8
ALU = mybir.AluOpType
ACTF = mybir.ActivationFunctionType

B, M, V, H, TD = 32, 512, 256, 1024, 10
NC = 8
RPC = M // NC              # 64 ranks per core
MEMROWS = B * M            # 16384
MEMP = MEMROWS + B         # 16416 gather-source rows (memory + x)
NKT = 2 * RPC              # 128 memory k-tiles per core
NBT = RPC * TD // 128      # 5 bits k-tiles per core
NTILES = NKT + NBT         # 133 live k-tiles
CHUNK = 8                  # k-tiles per W0 DMA
NIDX = RPC * B             # 2048 gather items
# fp8 region: each core's first RB local ranks; bf16 region: the rest
# (64-RB ranks) + 5 bits tiles + pad.
RB = 44                    # fp8 rank boundary (per-core local)
NKT8 = 2 * RB              # 88 fp8 k-tiles
NCH8 = NKT8 // CHUNK       # 11 fp8 chunks
NCHB = 6                   # bf16 chunks (40 mem + 5 bits + 3 pad)
W08_ROWS = NCH8 * CHUNK * 128    # 8192
W0B_ROWS = NCHB * CHUNK * 128    # 9216 (incl. 3 zero-pad tiles)
AS_SCALE = 8.0             # fp8 activation pre-scale
WS_SCALE = 256.0           # fp8 weight pre-scale
RING8 = 6                  # fp8 chunk ring depth (8KB/partition each)
RINGB = 3                  # bf16 chunk ring depth (16KB/partition each)
# Activation engine is slower per rank instr (Sign + accum read); give the
# DVE a third quadrant (mt=2) on the last batches so the Sign-normalized
# column ranges stay contiguous.
DVE3_B = {28, 29, 30, 31}

# constant packs (f32, 32 partitions): a = hot (keys chain), b = cold
_offs_a = {}
_wa = 0
for _name, _n in [("iota", M), ("timings", M), ("msur", M), ("gdata", M),
                  ("xrow", M), ("roff", 4), ("E0", 128), ("E1", 128)]:
    _offs_a[_name] = _wa
    _wa += _n
C32AW = _wa
_offs_b = {}
_wb = 0
for _name, _n in [("esel", B * 128), ("b1r", H), ("b2r", H), ("boutr", V)]:
    _offs_b[_name] = _wb
    _wb += _n
C32BW = _wb


def build_program(stage="full"):
    lvl = {"idx": 0, "gat": 1, "parth": 2, "full": 3}[stage]
    nc = bacc.Bacc(
        "TRN2",
        target_bir_lowering=False,
        debug=False,
        enable_asserts=False,
        num_devices=NC,
    )

    def din(name, shape, dtype=F32):
        return nc.dram_tensor(name, list(shape), dtype, kind="ExternalInput").ap()

    mem_plus = din("mem_plus", (MEMP, V), BF16)
    mem_plus8 = din("mem_plus8", (MEMP, V), F8)
    w0s8 = din("W0s8", (W08_ROWS, H), F8)
    w0s = din("W0s", (W0B_ROWS, H), BF16)
    w1 = din("W1", (H, H), BF16)
    w2 = din("W2", (H, H), BF16)
    wout = din("Wout", (H, V), BF16)
    c32a = din("c32a", (B, C32AW))
    c32b = din("c32b", (B, C32BW))
    c_eye = din("c_eye", (128, 128 + 8))
    c_eye16 = din("c_eye16", (128, 128), BF16)

    out = nc.dram_tensor("out", [B, V], F32, kind="ExternalOutput").ap()
    dbg = (nc.dram_tensor("dbg", [128, 256], F32, kind="ExternalOutput").ap()
           if stage != "full" else None)

    with tile.TileContext(nc) as tc:
        with (
            tc.tile_pool(name="const", bufs=1) as constp,
            tc.tile_pool(name="state", bufs=1) as state,
            tc.tile_pool(name="w0c8", bufs=RING8) as w0p8,
            tc.tile_pool(name="w0c", bufs=RINGB) as w0p,
            tc.tile_pool(name="pkrep", bufs=3, space="PSUM") as pkp,
            tc.tile_pool(name="ptr", bufs=2, space="PSUM") as ptrp,
            tc.tile_pool(name="ptrb", bufs=1, space="PSUM") as ptrbp,
            tc.tile_pool(name="ph", bufs=1, space="PSUM") as php,
            tc.tile_pool(name="dram", bufs=1, space="DRAM") as dramp,
        ):
            # ---- 4 const DMAs, then the W0 chunk stream (all on SP) --
            cpa = constp.tile([B, C32AW], F32, tag="cpa")
            nc.sync.dma_start(cpa[:], c32a)
            eyeb = constp.tile([128, 128 + 8], F32, tag="eye")
            nc.sync.dma_start(eyeb[:], c_eye)
            eye16 = constp.tile([128, 128], BF16, tag="eye16")
            nc.sync.dma_start(eye16[:], c_eye16)
            cpb = constp.tile([B, C32BW], F32, tag="cpb")
            nc.sync.dma_start(cpb[:], c32b)

            def ca(name, n):
                o = _offs_a[name]
                return cpa[:, o:o + n]

            def cb(name, n):
                o = _offs_b[name]
                return cpb[:, o:o + n]

            eye = eyeb[:, 0:128]
            b0T = eyeb[:, 128:136]
            esel = cb("esel", B * 128)
            iota = ca("iota", M)
            gdata = ca("gdata", M)
            xrow = ca("xrow", M)
            b1s = cb("b1r", H)
            b2s = cb("b2r", H)
            bouts = cb("boutr", V)
            t_sb = ca("timings", M)
            ms_sb = ca("msur", M)
            roff = ca("roff", 1)
            E0 = ca("E0", 128)
            E1 = ca("E1", 128)

            w08view = w0s8.rearrange("(kc j p) c -> kc p j c", p=128, j=CHUNK)
            w08tiles = []
            for kc in range(NCH8):
                w0t = w0p8.tile([128, CHUNK, H], F8, tag="w0c8")
                nc.sync.dma_start(w0t[:], w08view[kc])
                w08tiles.append(w0t)
            w0view = w0s.rearrange("(kc j p) c -> kc p j c", p=128, j=CHUNK)
            w0tiles = []
            for kc in range(NCHB):
                w0t = w0p.tile([128, CHUNK, H], BF16, tag="w0c")
                nc.sync.dma_start(w0t[:], w0view[kc])
                w0tiles.append(w0t)

            # ---- stage A: argmin slot + keys + scatter data ----------
            msur2 = state.tile([B, M], F32, tag="msur2")
            nc.vector.tensor_scalar(msur2[:], ms_sb, 0.9, None, ALU.mult)
            minv = state.tile([B, 1], F32, tag="minv")
            nc.vector.tensor_reduce(minv[:], msur2[:], axis=mybir.AxisListType.X,
                                    op=ALU.min)
            mask = state.tile([B, M], U8, tag="mask")
            nc.vector.tensor_scalar(mask[:], msur2[:], minv[:], None, ALU.is_equal)
            cand = state.tile([B, M], F32, tag="cand")
            nc.vector.memset(cand[:], 1.0e9)
            nc.vector.copy_predicated(cand[:], mask[:], iota)
            idx0 = state.tile([B, 1], F32, tag="idx0")
            nc.vector.tensor_reduce(idx0[:], cand[:], axis=mybir.AxisListType.X,
                                    op=ALU.min)

            keys = state.tile([B, M], F32, tag="keys")
            nc.vector.tensor_scalar(keys[:], t_sb, 512.0, 512.0, ALU.mult, ALU.add)
            nc.vector.tensor_tensor(keys[:], keys[:], iota, ALU.add)
            mask2 = state.tile([B, M], U8, tag="mask2")
            nc.vector.tensor_scalar(mask2[:], iota, idx0[:], None, ALU.is_equal)
            nc.vector.copy_predicated(keys[:], mask2[:], iota)

            # gather data values (row ids), override slot -> x row; built
            # with gpsimd arithmetic, off the DVE critical path:
            # gd = gdata + m2f * (xrow - gdata)
            m2f = state.tile([B, M], F32, tag="m2f")
            nc.gpsimd.tensor_scalar(m2f[:], iota, idx0[:], None, ALU.is_equal)
            gdd = state.tile([B, M], F32, tag="gdd")
            nc.gpsimd.tensor_tensor(gdd[:], xrow, gdata, ALU.subtract)
            nc.gpsimd.tensor_tensor(gdd[:], gdd[:], m2f[:], ALU.mult)
            gd = state.tile([B, M], F32, tag="gd")
            nc.gpsimd.tensor_tensor(gd[:], gdd[:], gdata, ALU.add)
            gd16 = state.tile([B, M], I16, tag="gd16")
            nc.gpsimd.tensor_copy(gd16[:], gd[:])
            # effective timings (t+1, overridden slot -> 0) for the (late)
            # bits tiles: teff = (t+1) * (1 - m2f), via gpsimd as well
            teff = state.tile([B, M], F32, tag="teff")
            nc.scalar.activation(teff[:], t_sb, ACTF.Copy, bias=1.0)
            ct = state.tile([B, M], F32, tag="ct")
            nc.gpsimd.tensor_scalar(ct[:], m2f[:], -1.0, 1.0, ALU.mult, ALU.add)
            nc.gpsimd.tensor_tensor(teff[:], teff[:], ct[:], ALU.mult)
            st16 = state.tile([B, M], F16, tag="st16")
            nc.gpsimd.tensor_copy(st16[:], teff[:])

            # ---- keysT via PE transpose ------------------------------
            keysT = state.tile([128, 4 * B], F32, tag="keysT")
            for mt in range(4):
                ptt = ptrp.tile([128, 128], F32, tag="pm")
                nc.tensor.transpose(ptt[:, 0:B], keys[:, mt * 128:(mt + 1) * 128],
                                    eye[0:B, 0:B])
                nc.scalar.activation(keysT[:, mt * B:(mt + 1) * B], ptt[:, 0:B],
                                     ACTF.Copy)

            # ---- ranks: brute force split DVE / Activation -----------
            rank_sb = state.tile([128, 4 * B], F32, tag="rank")
            scr_v = state.tile([128, M], BF16, tag="scr_v")
            scr_a = state.tile([128, M], BF16, tag="scr_a")
            for b in range(B):
                pk_t = pkp.tile([128, M], F32, tag="pkrep")
                nc.tensor.matmul(pk_t[:], esel[:, b * 128:(b + 1) * 128], keys[:],
                                 start=True, stop=True)
                ndve = 3 if b in DVE3_B else 2
                for mt in range(4):
                    col = mt * B + b
                    sc = keysT[:, col:col + 1]
                    if mt < ndve:
                        nc.vector.tensor_scalar(
                            scr_v[:], pk_t[:], sc, None, ALU.is_lt, ALU.add,
                            accum_out=rank_sb[:, col:col + 1])
                    else:
                        nc.scalar.activation(
                            scr_a[:], pk_t[:], ACTF.Sign, bias=sc, scale=-1.0,
                            accum_out=rank_sb[:, col:col + 1])
            # Sign-engine columns: rank = (S + 511) / 2 (contiguous ranges)
            nb3 = len(DVE3_B)
            nc.vector.tensor_scalar(rank_sb[:, 2 * B:3 * B - nb3],
                                    rank_sb[:, 2 * B:3 * B - nb3],
                                    0.5, 255.5, ALU.mult, ALU.add)
            nc.vector.tensor_scalar(rank_sb[:, 3 * B:4 * B],
                                    rank_sb[:, 3 * B:4 * B],
                                    0.5, 255.5, ALU.mult, ALU.add)

            # ---- rank -> [B, M] layout via PE transpose --------------
            rankT = state.tile([B, M], F32, tag="rankT")
            for mt in range(4):
                ptt = ptrp.tile([128, 128], F32, tag="pm")
                nc.tensor.transpose(ptt[0:B, 0:128],
                                    rank_sb[:, mt * B:(mt + 1) * B], eye)
                nc.scalar.activation(rankT[:, mt * 128:(mt + 1) * 128],
                                     ptt[0:B, 0:128], ACTF.Copy)

            # window: idxs = (rank - 64c) if in [0,64) else negative
            ri = state.tile([B, M], F32, tag="ri")
            nc.vector.tensor_scalar(ri[:], rankT[:], roff, None, ALU.subtract)
            tmask = state.tile([B, M], F32, tag="tmask")
            nc.vector.tensor_scalar(tmask[:], ri[:], 64.0, None, ALU.is_lt)
            nc.vector.tensor_tensor(ri[:], ri[:], tmask[:], ALU.mult)
            nc.vector.tensor_tensor(ri[:], ri[:], tmask[:], ALU.add)
            sidx = state.tile([B, M], I16, tag="sidx")
            nc.vector.tensor_scalar(sidx[:], ri[:], 1.0, None, ALU.subtract)

            # ---- local_scatter: invert permutation -------------------
            idxc = state.tile([B, RPC], I16, tag="idxc")
            nc.gpsimd.local_scatter(idxc[:], gd16[:], sidx[:],
                                    channels=B, num_elems=RPC, num_idxs=M)

            # ---- gather idx tile via PE selector broadcast -----------
            # idxfull[p, f] = idxc[p%16 + 16*(f%2), f//2], replicated over
            # the 8 gpsimd cores (p//16).
            idxf = state.tile([B, RPC], F32, tag="idxf")
            nc.vector.tensor_copy(idxf[:], idxc[:])
            Y = state.tile([B, 2, 128], F32, tag="Y")
            nc.vector.memset(Y[:], 0.0)
            yv = Y.rearrange("p two (r par) -> p two r par", par=2)
            nc.vector.tensor_copy(yv[:, 0, :, 0], idxf[:])
            nc.vector.tensor_copy(yv[:, 1, :, 1], idxf[:])
            pidx = ptrp.tile([128, 128], F32, tag="pm")
            nc.tensor.matmul(pidx[:], E0, Y[:, 0, :], start=True, stop=False)
            nc.tensor.matmul(pidx[:], E1, Y[:, 1, :], start=False, stop=True)
            idxfull = state.tile([128, 128], I16, tag="idxfull")
            nc.vector.tensor_copy(idxfull[:], pidx[:])

            if stage == "idx":
                tmpd = state.tile([128, 128], F32, tag="tmpd")
                nc.vector.tensor_copy(tmpd[:], idxfull[:])
                nc.scalar.dma_start(dbg[:, 0:128], tmpd[:])
                stc16d = state.tile([B, RPC], F16, tag="stc16d")
                nc.gpsimd.local_scatter(stc16d[:], st16[:], sidx[:],
                                        channels=B, num_elems=RPC, num_idxs=M)
                tmpd2 = state.tile([B, RPC], F32, tag="tmpd2")
                nc.vector.tensor_copy(tmpd2[:], stc16d[:])
                nc.scalar.dma_start(dbg[0:B, 128:128 + RPC], tmpd2[:])
            try:
                if lvl < 1:
                    raise _SkipRest

                # ---- transposed gathers: all 128 lhsT tiles ----------
                # ranks [0,RB) from the fp8 (pre-scaled) copy, rest bf16.
                N8 = RB * B
                NB16 = NIDX - N8
                G8 = state.tile([128, 2, N8], F8, tag="G8")
                G1 = state.tile([128, 2, NB16], BF16, tag="G1")
                nc.gpsimd.dma_gather(
                    out_ap=G8[:], in_ap=mem_plus8, idxs_ap=idxfull[:, 0:2 * RB],
                    num_idxs=N8, num_idxs_reg=N8,
                    elem_size=V, transpose=True, single_packet=False)
                nc.gpsimd.dma_gather(
                    out_ap=G1[:], in_ap=mem_plus, idxs_ap=idxfull[:, 2 * RB:128],
                    num_idxs=NB16, num_idxs_reg=NB16,
                    elem_size=V, transpose=True, single_packet=False)
                # sorted-timings scatter only feeds the (late) bits tiles
                stc16 = state.tile([B, RPC], F16, tag="stc16")
                nc.gpsimd.local_scatter(stc16[:], st16[:], sidx[:],
                                        channels=B, num_elems=RPC, num_idxs=M)
                # fp8 transpose is 16-bit granular: free byte 2i+e of
                # partition p holds elem v=2p+e of item i.
                G8i = (G8[:].rearrange("p two i -> p (two i)")
                       .rearrange("p (i e) -> p i e", e=2))

                # ---- bits for core's 64 ranks ------------------------
                stc = state.tile([B, RPC], F32, tag="stc")
                nc.vector.tensor_copy(stc[:], stc16[:])
                u_all = state.tile([B, RPC, TD], F32, tag="u_all")
                tmpu = state.tile([B, RPC], F32, tag="tmpu")
                rem = stc
                for j in range(9, -1, -1):
                    ud = u_all[:, :, j]
                    nc.vector.tensor_scalar(ud, rem[:], float(2 ** j), None, ALU.is_ge)
                    nc.vector.tensor_scalar(tmpu[:], ud, float(2 ** j), None, ALU.mult)
                    nc.vector.tensor_tensor(rem[:], rem[:], tmpu[:], ALU.subtract)
                u16 = state.tile([B, RPC * TD], BF16, tag="u16")
                nc.vector.tensor_copy(u16[:], u_all.rearrange("p r d -> p (r d)")[:])
                bitsT = state.tile([128, NBT * B], BF16, tag="bitsT")
                for tt in range(NBT):
                    ptb = ptrbp.tile([128, 256], BF16, tag="pmb")
                    nc.tensor.transpose(ptb[:, 0:B],
                                        u16[:, tt * 128:(tt + 1) * 128],
                                        eye16[0:B, 0:B])
                    nc.scalar.activation(bitsT[:, tt * B:(tt + 1) * B],
                                         ptb[:, 0:B], ACTF.Copy)

                if stage == "gat":
                    tmpg = state.tile([128, 256], F32, tag="tmpg")
                    nc.vector.tensor_copy(
                        tmpg[:, 0:128],
                        G8[:].rearrange("p two i -> p (two i)")[:, 0:128])
                    nc.vector.tensor_copy(tmpg[:, 128:160], bitsT[:, 0:32])
                    nc.vector.tensor_copy(tmpg[:, 160:256], G1[:, 1, 0:96])
                    nc.scalar.dma_start(dbg[:, :], tmpg[:])
                if lvl < 2:
                    raise _SkipRest

                # ---- main matmul pass 1: fp8 ranks 0-31 --------------
                ph_t = php.tile([B, H], F32, tag="ph")
                for kt in range(NKT8):
                    w0t = w08tiles[kt // CHUNK]
                    j = kt % CHUNK
                    r, e = kt // 2, kt % 2
                    lhsT = G8i[:, B * r:B * (r + 1), e]
                    last = kt == NKT8 - 1
                    nc.tensor.matmul(ph_t[:, 0:512], lhsT, w0t[:, j, 0:512],
                                     start=(kt == 0), stop=last)
                    nc.tensor.matmul(ph_t[:, 512:1024], lhsT, w0t[:, j, 512:1024],
                                     start=(kt == 0), stop=last)
                h8 = state.tile([B, H], F32, tag="h8")
                nc.vector.tensor_scalar(h8[:], ph_t[:],
                                        1.0 / (AS_SCALE * WS_SCALE), None, ALU.mult)

                # ---- main matmul pass 2: bf16 ranks 32-63 + bits -----
                ph_t2 = php.tile([B, H], F32, tag="ph")
                nbt_tiles = NTILES - NKT8   # 69
                for ktb in range(nbt_tiles):
                    w0t = w0tiles[ktb // CHUNK]
                    j = ktb % CHUNK
                    kt = NKT8 + ktb
                    if kt < NKT:
                        r, hh = kt // 2, kt % 2
                        lhsT = G1[:, hh, B * (r - RB):B * (r - RB + 1)]
                    else:
                        tb = kt - NKT
                        lhsT = bitsT[:, tb * B:(tb + 1) * B]
                    last = ktb == nbt_tiles - 1
                    nc.tensor.matmul(ph_t2[:, 0:512], lhsT, w0t[:, j, 0:512],
                                     start=(ktb == 0), stop=last)
                    nc.tensor.matmul(ph_t2[:, 512:1024], lhsT, w0t[:, j, 512:1024],
                                     start=(ktb == 0), stop=last)

                part_h = state.tile([B, H], F32, tag="part_h")
                nc.vector.tensor_tensor(part_h[:], ph_t2[:], h8[:], ALU.add)
                if stage == "parth":
                    nc.scalar.dma_start(dbg[0:B, 0:256], part_h[:, 0:256])
                if lvl < 3:
                    raise _SkipRest

                # ---- AllReduce partial h over the 8 cores ------------
                cc_in = dramp.tile([B, H], F32, tag="cc_in")
                cc_out = dramp.tile([B, H], F32, tag="cc_out")
                nc.sync.dma_start(cc_in[:], part_h[:])
                nc.gpsimd.collective_compute(
                    "AllReduce", ALU.add,
                    replica_groups=[list(range(NC))],
                    ins=[cc_in.opt()],
                    outs=[cc_out.opt()],
                )
                h_sb = state.tile([B, H], F32, tag="h_sb")
                nc.sync.dma_start(h_sb[:], cc_out[:])

                # ---- dense tail (replicated, bf16 weights) -----------
                # relu (+ l1 bias) is fused into the transposed-copy: the
                # hT copy applies Relu(in + bias_col) on the Act engine.
                def dense(h_in, w_dram, bias_T, n_out, tag):
                    wt = w0p.tile([128, CHUNK, H], BF16, tag="w0c")
                    wv = w_dram.rearrange("(j p) c -> p j c", p=128, j=CHUNK)
                    nc.sync.dma_start(wt[:, :, 0:n_out], wv)
                    hT = state.tile([128, 8 * B], BF16, tag=f"hT_{tag}")
                    for kt in range(8):
                        ptt = ptrp.tile([128, 128], F32, tag="pm")
                        nc.tensor.transpose(ptt[:, 0:B],
                                            h_in[:, kt * 128:(kt + 1) * 128],
                                            eye[0:B, 0:B])
                        bcol = (bias_T[:, kt:kt + 1] if bias_T is not None
                                else 0.0)
                        nc.scalar.activation(hT[:, kt * B:(kt + 1) * B],
                                             ptt[:, 0:B], ACTF.Relu, bias=bcol)
                    pho = php.tile([B, n_out], F32, tag="ph")
                    for kt in range(8):
                        for j0 in range(0, n_out, 512):
                            jn = min(512, n_out - j0)
                            nc.tensor.matmul(
                                pho[:, j0:j0 + jn], hT[:, kt * B:(kt + 1) * B],
                                wt[:, kt, j0:j0 + jn],
                                start=(kt == 0), stop=(kt == 7))
                    return pho

                pho1 = dense(h_sb, w1, b0T, H, "l1")
                h1 = state.tile([B, H], F32, tag="h_l1")
                nc.vector.tensor_tensor(h1[:], pho1[:], b1s, ALU.add)
                pho2 = dense(h1, w2, None, H, "l2")
                h2 = state.tile([B, H], F32, tag="h_l2")
                nc.vector.tensor_tensor(h2[:], pho2[:], b2s, ALU.add)
                pho3 = dense(h2, wout, None, V, "lo")
                logits = state.tile([B, V], F32, tag="h_lo")
                nc.vector.tensor_tensor(logits[:], pho3[:], bouts, ALU.add)
                nc.sync.dma_start(out, logits[:])
            except _SkipRest:
                pass

    nc.compile()
    return nc


def make_in_maps(inputs):
    x = np.asarray(inputs["x"], np.float32)
    memory = np.asarray(inputs["memory"], np.float32)
    timings = np.asarray(inputs["memory_timings"], np.float32)
    msur = np.asarray(inputs["memory_surprise"], np.float32)
    W0 = np.asarray(inputs["W0"], np.float32)
    W1 = np.asarray(inputs["W1"], np.float32)
    W2 = np.asarray(inputs["W2"], np.float32)
    Wout = np.asarray(inputs["Wout"], np.float32)
    b0 = np.asarray(inputs["b0"], np.float32)
    b1 = np.asarray(inputs["b1"], np.float32)
    b2 = np.asarray(inputs["b2"], np.float32)
    bout = np.asarray(inputs["bout"], np.float32)

    bf = ml_dtypes.bfloat16
    f8 = ml_dtypes.float8_e4m3
    mem_plus_f = np.concatenate([memory.reshape(MEMROWS, V), x], 0)
    mem_plus = mem_plus_f.astype(bf)
    mem_plus8 = (mem_plus_f.astype(bf).astype(np.float32)
                 * AS_SCALE).astype(f8)
    W1b = np.ascontiguousarray(W1.astype(bf))
    W2b = np.ascontiguousarray(W2.astype(bf))
    Woutb = np.ascontiguousarray(Wout.astype(bf))

    iota512 = np.broadcast_to(np.arange(M, dtype=np.float32), (B, M))
    gdata = 512.0 * np.arange(B, dtype=np.float32)[:, None] + iota512
    xrowc = np.broadcast_to(
        (MEMROWS + np.arange(B, dtype=np.float32))[:, None], (B, M))
    esel = np.zeros((B, B * 128), np.float32)
    for b in range(B):
        esel[b, b * 128:(b + 1) * 128] = 1.0
    p = np.arange(128)
    E0 = (np.arange(B)[:, None] == (p % 16)[None, :]).astype(np.float32)
    E1 = (np.arange(B)[:, None] == (p % 16 + 16)[None, :]).astype(np.float32)

    def pack_a(core):
        c = np.zeros((B, C32AW), np.float32)

        def put(name, arr):
            o = _offs_a[name]
            c[:, o:o + arr.shape[1]] = arr

        put("iota", iota512)
        put("timings", timings)
        put("msur", msur)
        put("gdata", gdata)
        put("xrow", xrowc)
        put("roff", np.full((B, 4), 64.0 * core, np.float32))
        put("E0", E0)
        put("E1", E1)
        return c

    cb = np.zeros((B, C32BW), np.float32)
    cb[:, _offs_b["esel"]:_offs_b["esel"] + B * 128] = esel
    cb[:, _offs_b["b1r"]:_offs_b["b1r"] + H] = np.broadcast_to(b1, (B, H))
    cb[:, _offs_b["b2r"]:_offs_b["b2r"] + H] = np.broadcast_to(b2, (B, H))
    cb[:, _offs_b["boutr"]:_offs_b["boutr"] + V] = np.broadcast_to(bout, (B, V))

    eyepack = np.zeros((128, 128 + 8), np.float32)
    eyepack[:, 0:128] = np.eye(128, dtype=np.float32)
    eyepack[:, 128:136] = b0.reshape(8, 128).T

    shared = {
        "mem_plus": mem_plus,
        "mem_plus8": mem_plus8,
        "W1": W1b, "W2": W2b, "Wout": Woutb,
        "c32b": cb,
        "c_eye": eyepack,
        "c_eye16": np.eye(128, dtype=np.float32).astype(bf),
    }

    in_maps = []
    half = RB * V           # fp8 W0 rows per core (ranks [0, RB))
    for core in range(NC):
        base = core * RPC * V
        # fp8 region: ranks [0,RB), rows pair-transposed so k-tile (2r+e, p)
        # maps to original row 256r + 2p + e (16-bit-granular gather layout)
        A = W0[base:base + half].reshape(RB, 128, 2, H)
        w0s8 = np.ascontiguousarray(
            (A.transpose(0, 2, 1, 3).reshape(W08_ROWS, H) * WS_SCALE)
        ).astype(f8)
        # bf16 region: ranks [RB,64) + bits rows + zero pad
        nb = (RPC - RB) * V     # 5120
        w0shard = np.zeros((W0B_ROWS, H), bf)
        w0shard[:nb] = W0[base + half:base + RPC * V].astype(bf)
        w0shard[nb:nb + RPC * TD] = (
            W0[M * V + core * RPC * TD: M * V + (core + 1) * RPC * TD]
            .astype(bf))
        m = dict(shared)
        m["W0s8"] = w0s8
        m["W0s"] = w0shard
        m["c32a"] = pack_a(core)
        in_maps.append(m)
    return in_maps


_NC_CACHE = None


def kernel(**inputs) -> np.ndarray:
    global _NC_CACHE
    if _NC_CACHE is None:
        _NC_CACHE = build_program()
    nc = _NC_CACHE
    in_maps = make_in_maps(inputs)
    res = bass_utils.run_bass_kernel_spmd(nc, in_maps, core_ids=list(range(NC)))
    return np.asarray(res.results[0]["out"], np.float32)


if __name__ == "__main__":
    np.random.seed(0)
    build_program()
    print("build OK")
